# revision 26
# baseline (speedup 1.0000x reference)
"""GatingAttentionLayerWsa on 8 TRN2 NeuronCores.

Shapes: B=4, S=L=2048, E=512, H=8, D=64.

Sharding: core c = (batch b=c//2, query-half c%2). Each core holds the FULL
K/V for its batch (projections duplicated across the pair) and only its half
of the query rows (L/2=1024). It computes attention for all 8 heads over its
query rows and the full out-projection y = o @ Wo + bo for those rows. No
collective is needed; the host concatenates the two L-halves per batch.

Math notes:
 - Row stats are algebraic: var_l = q_l^T Cc q_l with Cc = K^T K/S - km km^T
   (centered covariance), so no reduction over the 2048-wide logit rows.
 - softmax((z-mu)/std) == softmax(z/std): only r = 1/std is applied, folded
   into q before the QK^T matmul (q~ = q * r).
 - Scores are computed transposed (Z^T [S_part, L_free]) so P^T = exp(Z^T)
   feeds the PV matmul directly (lhsT = [v | 1], moving = P^T). The ones
   column makes row 64 accumulate the softmax row-sums; the final per-row
   normalization scales the small [64, L] output.
 - Projections run in float32r; the attention operands (q~, k, v, P) are
   bf16 with fp32 PSUM accumulation. 1/x uses the fast Newton-Raphson
   approximation (~18 bits), not the slow iterative divide.

ISA constraints honored: matmul lhsT/rhs share a base partition; DVE
tensor-tensor with both inputs in SBUF needs equal bases (PSUM inputs are
exempt); gpsimd partition_broadcast needs a base-0 [1,N] source and a
full-tile destination; matmul moving operands are <=512 columns; engine
partition offsets are multiples of 32.

For timing, `_build(reps>1)` wraps the whole body in a hardware For_i loop
(the back-edge is a full barrier, so reps serialize) — instruction count
stays flat, so large rep counts compile quickly and the (t_R - t_1)/(R-1)
delta dwarfs host-side dispatch noise.
"""
import sys
from contextlib import ExitStack

import numpy as np

try:
    import concourse.bass as bass  # noqa: F401
except ImportError:  # pragma: no cover
    sys.path.insert(0, "/opt/trn_rl_repo")

import concourse.bacc as bacc
import concourse.mybir as mybir
import concourse.tile as tile
from concourse import masks
from concourse.bass_utils import run_bass_kernel_spmd

B, S, E, H, D = 4, 2048, 512, 8, 64
LH = S // 2        # 1024 query rows per core
N_CORES = 8
D1 = D + 1         # head slot width incl ones column
KW = H * D1        # 520: k/v natural width with interleaved ones
NSC = S // 128     # 16 chunks of 128 along S
NQC = LH // 128    # 8 chunks of 128 along the L-half
NEC = E // 128     # 4 chunks of 128 along E
NT = 4             # head-pair chunks (128 rows of DG each)
F32 = mybir.dt.float32
F32R = mybir.dt.float32r
BF16 = mybir.dt.bfloat16
AF = mybir.ActivationFunctionType

_CACHE = {}


def _build(reps=1, debug=False):
    nc = bacc.Bacc("TRN2", target_bir_lowering=False, debug=False,
                   num_devices=N_CORES)
    dbg_out = {}
    if debug:
        for nm, shape, dt in [
                ("d_knat", [128, KW], BF16), ("d_vsb", [128, KW], BF16),
                ("d_cc", [64, D], BF16), ("d_qt", [64, LH], BF16),
                ("d_srow", [1, LH], F32), ("d_rrow", [1, LH], F32),
                ("d_rb", [64, LH], F32), ("d_kt", [64, S], BF16),
                ("d_ot", [128, LH], F32), ("d_rs", [1, LH], F32),
                ("d_ri", [1, LH], F32), ("d_ib", [64, LH], F32)]:
            dbg_out[nm] = nc.dram_tensor(nm, shape, dt,
                                         kind="ExternalOutput").ap()
    src_q = nc.dram_tensor("src_q", [LH, E], F32, kind="ExternalInput").ap()
    src_k = nc.dram_tensor("src_k", [S, E], F32, kind="ExternalInput").ap()
    src_v = nc.dram_tensor("src_v", [S, E], F32, kind="ExternalInput").ap()
    wq = nc.dram_tensor("wq", [E, E], F32, kind="ExternalInput").ap()
    wk = nc.dram_tensor("wk", [E, E], F32, kind="ExternalInput").ap()
    wv = nc.dram_tensor("wv", [E, E], F32, kind="ExternalInput").ap()
    wo = nc.dram_tensor("wo", [E, E], F32, kind="ExternalInput").ap()
    bq = nc.dram_tensor("bq", [1, E], F32, kind="ExternalInput").ap()
    bk = nc.dram_tensor("bk", [1, E], F32, kind="ExternalInput").ap()
    bv = nc.dram_tensor("bv", [1, E], F32, kind="ExternalInput").ap()
    bo = nc.dram_tensor("bo", [1, E], F32, kind="ExternalInput").ap()
    out = nc.dram_tensor("out", [LH, E], F32, kind="ExternalOutput").ap()

    with tile.TileContext(nc) as tc, ExitStack() as X:
        sb = X.enter_context(tc.tile_pool(name="sb", bufs=1))

        # ---- constants (once) ----
        identf = sb.tile([128, 128], F32)
        masks.make_identity(nc, identf[:])
        identr_t = sb.tile([128, 128], F32R)
        nc.vector.tensor_copy(identr_t[:], identf[:])
        identr = identr_t[:]
        identb = sb.tile([128, 128], BF16)
        nc.vector.tensor_copy(identb[:], identf[:])
        ones64b = sb.tile([64, 1], BF16)
        nc.gpsimd.memset(ones64b[:], 1.0)
        ones8 = sb.tile([128, H], BF16)
        nc.gpsimd.memset(ones8[:], 1.0)
        epsb = sb.tile([1, 1], F32)
        nc.gpsimd.memset(epsb[:], 1e-6)

        def body():
            with ExitStack() as XR:
                pr = XR.enter_context(tc.tile_pool(name="pr", bufs=1))
                scr = XR.enter_context(
                    tc.tile_pool(name="scr", bufs=1, space="PSUM"))

                # ---- weights & biases ----
                def load_w(name, src):
                    ts = []
                    for e in range(NEC):
                        t = pr.tile([128, E], F32R, name=f"{name}{e}",
                                    tag=f"{name}{e}")
                        nc.sync.dma_start(
                            t[:], src[e * 128:(e + 1) * 128, :].bitcast(F32R))
                        ts.append(t)
                    return ts

                wk_t = load_w("wk", wk)
                wq_t = load_w("wq", wq)
                wv_t = load_w("wv", wv)
                wo_t = load_w("wo", wo)

                def bcast_bias(name, src):
                    row = pr.tile([1, E], F32, name=f"{name}_row",
                                  tag=f"{name}_row")
                    nc.sync.dma_start(row[:], src[:])
                    full = pr.tile([128, E], F32, name=f"{name}_b",
                                   tag=f"{name}_b")
                    nc.gpsimd.partition_broadcast(full[:], row[:])
                    return full

                bkb = bcast_bias("bk", bk)
                bvb = bcast_bias("bv", bv)
                bob = bcast_bias("bo", bo)
                bqc = []
                for t in range(NT):
                    c = pr.tile([128, 1], F32, name=f"bqc{t}", tag=f"bqc{t}")
                    nc.sync.dma_start(
                        c[:],
                        bq[0:1, t * 128:(t + 1) * 128].rearrange("a b -> b a"))
                    bqc.append(c)

                def stream_groups(pool, src, nrows, who, consume):
                    """Per 512-row group: load nat chunks, transpose to
                    sg[e] [128, 512] (X^T slices), then consume(g, sg)."""
                    ngrp = nrows // 512
                    for g in range(ngrp):
                        nats = []
                        for i in range(4):
                            sc = g * 4 + i
                            nat = pool.tile([128, E], F32R,
                                            name=f"nat{who}{sc}",
                                            tag="nat", bufs=6)
                            nc.sync.dma_start(
                                nat[:],
                                src[sc * 128:(sc + 1) * 128, :].bitcast(F32R))
                            nats.append(nat)
                        sg = []
                        for e in range(NEC):
                            pt = scr.tile([128, 512], F32R,
                                          name=f"pt{who}{g}_{e}", tag="scr",
                                          bufs=2)
                            for i in range(4):
                                nc.tensor.transpose(
                                    pt[:, i * 128:(i + 1) * 128],
                                    nats[i][:, e * 128:(e + 1) * 128],
                                    identr)
                            sgt = pool.tile([128, 512], F32R,
                                            name=f"sg{who}{g}_{e}",
                                            tag=f"sg{e}", bufs=2)
                            if who == "k":
                                nc.scalar.activation(sgt[:], pt[:], AF.Copy)
                            else:
                                nc.vector.tensor_copy(sgt[:], pt[:])
                            sg.append(sgt)
                        consume(g, sg)

                def proj_nat_consume(w_t, bias_b, dst, who):
                    """dst[sc] [128, KW] bf16 = [x@W + b | 1] interleaved."""
                    def consume(g, sg):
                        for i in range(4):
                            sc = g * 4 + i
                            pp = scr.tile([128, E], F32, name=f"pp{who}{sc}",
                                          tag="scr", bufs=2)
                            for e in range(NEC):
                                nc.tensor.matmul(
                                    pp[:], sg[e][:, i * 128:(i + 1) * 128],
                                    w_t[e][:], start=(e == 0),
                                    stop=(e == NEC - 1))
                            d3 = dst[sc][:].rearrange("p (h w) -> p h w", h=H)
                            nc.vector.tensor_add(
                                d3[:, :, 0:D],
                                pp[:].rearrange("p (h w) -> p h w", h=H),
                                bias_b[:].rearrange("p (h w) -> p h w", h=H))
                            nc.vector.tensor_copy(
                                d3[:, :, D:D1],
                                ones8[:].rearrange("p (h w) -> p h w", h=H))
                    return consume

                # ---- K path ----
                k_nat = [pr.tile([128, KW], BF16, name=f"kn{sc}",
                                 tag=f"kn{sc}") for sc in range(NSC)]
                with tc.tile_pool(name="pk", bufs=1) as pk:
                    stream_groups(pk, src_k, S, "k",
                                  proj_nat_consume(wk_t, bkb, k_nat, "k"))
                if debug:
                    nc.sync.dma_start(dbg_out["d_knat"][:], k_nat[0][:])

                # ---- centered covariance per head: cc = K^T K/S - km km^T --
                cc = [pr.tile([64, D], BF16, name=f"cc{h}", tag=f"cc{h}")
                      for h in range(H)]
                with tc.tile_pool(name="pcp", bufs=1, space="PSUM") as pcp:
                    for h in range(H):
                        ka = slice(h * D1, (h + 1) * D1)
                        pc = pcp.tile([D1, D1], F32, name=f"pc{h}", tag="pc")
                        for sc in range(NSC):
                            nc.tensor.matmul(pc[:], k_nat[sc][:, ka],
                                             k_nat[sc][:, ka],
                                             start=(sc == 0),
                                             stop=(sc == NSC - 1))
                        ckm = pr.tile([D1, D1], BF16, name=f"ckm{h}",
                                      tag="ckm", bufs=2)
                        nc.vector.tensor_scalar_mul(ckm[:], pc[:], 1.0 / S)
                        oo = pcp.tile([D, D], F32, name=f"oo{h}", tag="oo")
                        nc.tensor.matmul(oo[:], ckm[D:D1, 0:D],
                                         ckm[D:D1, 0:D], start=True, stop=True)
                        nc.vector.tensor_sub(cc[h][:], ckm[0:D, 0:D], oo[:])
                if debug:
                    nc.sync.dma_start(dbg_out["d_cc"][:], cc[0][:])

                # ---- Q path: transposed projection ----
                qT = [pr.tile([64, LH], BF16, name=f"qT{h}", tag=f"qT{h}")
                      for h in range(H)]

                def q_consume(g, sg):
                    js = slice(g * 512, (g + 1) * 512)
                    for t in range(NT):
                        pp = scr.tile([128, 512], F32, name=f"ppq{t}{g}",
                                      tag="scr", bufs=2)
                        for e in range(NEC):
                            nc.tensor.matmul(
                                pp[:], wq_t[e][:, t * 128:(t + 1) * 128],
                                sg[e][:], start=(e == 0), stop=(e == NEC - 1))
                        nc.vector.tensor_scalar_add(
                            qT[2 * t][:, js], pp[0:64, :], bqc[t][0:64, :])
                        nc.vector.tensor_scalar_add(
                            qT[2 * t + 1][:, js], pp[64:128, :],
                            bqc[t][64:128, :])

                with tc.tile_pool(name="pq", bufs=1) as pq:
                    stream_groups(pq, src_q, LH, "q", q_consume)

                # ---- row stats + fold r into q~ ----
                for h in range(H):
                    srow = pr.tile([1, LH], F32, name=f"srow{h}", tag="brow",
                                   bufs=4)
                    for j in range(2):
                        js = slice(j * 512, (j + 1) * 512)
                        pu = scr.tile([64, 512], F32, name=f"pu{h}{j}",
                                      tag="scr", bufs=2)
                        nc.tensor.matmul(pu[:], cc[h][:], qT[h][:, js],
                                         start=True, stop=True)
                        wb = pr.tile([64, 512], BF16, name=f"wb{h}{j}",
                                     tag="wb", bufs=2)
                        nc.vector.tensor_mul(wb[:], pu[:], qT[h][:, js])
                        pv = scr.tile([1, 512], F32, name=f"pvar{h}{j}",
                                      tag="scr", bufs=2)
                        nc.tensor.matmul(pv[:], ones64b[:], wb[:],
                                         start=True, stop=True)
                        # srow = sqrt(var + 1e-6), straight from PSUM
                        nc.scalar.activation(srow[:, js], pv[:], AF.Sqrt,
                                             bias=epsb[:], scale=1.0)
                    nc.vector.tensor_scalar_add(srow[:], srow[:], 1e-6)
                    rrow = pr.tile([1, LH], F32, name=f"rrow{h}", tag="brow",
                                   bufs=4)
                    nc.vector.reciprocal_approx_fast(rrow[:], srow[:])
                    rb = pr.tile([64, LH], F32, name=f"rb{h}", tag="bcol",
                                 bufs=2)
                    nc.gpsimd.partition_broadcast(rb[:], rrow[:])
                    nc.vector.tensor_mul(qT[h][:], qT[h][:], rb[:])
                    if debug and h == 0:
                        nc.sync.dma_start(dbg_out["d_srow"][:], srow[:])
                        nc.sync.dma_start(dbg_out["d_rrow"][:], rrow[:])
                        nc.sync.dma_start(dbg_out["d_rb"][:], rb[:])
                        nc.sync.dma_start(dbg_out["d_qt"][:], qT[0][:])

                # ---- V path ----
                v_sb = [pr.tile([128, KW], BF16, name=f"vn{sc}",
                                tag=f"vn{sc}") for sc in range(NSC)]
                with tc.tile_pool(name="pv", bufs=1) as pvp:
                    stream_groups(pvp, src_v, S, "v",
                                  proj_nat_consume(wv_t, bvb, v_sb, "v"))
                if debug:
                    nc.sync.dma_start(dbg_out["d_vsb"][:], v_sb[0][:])

                # ---- attention (per head) + out projection ----
                kT = [pr.tile([64, S], BF16, name=f"kT{h}", tag=f"kT{h}")
                      for h in range(H)]
                oTp = [pr.tile([128, LH], F32R, name=f"oT{t}", tag=f"oT{t}")
                       for t in range(NT)]
                with tc.tile_pool(name="pat", bufs=1, space="PSUM") as pat:
                    for h in range(H):
                        t, rh = h // 2, (h % 2) * 64
                        ks = slice(h * D1, h * D1 + D)
                        for scg in range(0, NSC, 4):
                            ptk = scr.tile([64, 512], BF16,
                                           name=f"ptk{h}_{scg}", tag="scr",
                                           bufs=2)
                            for i in range(4):
                                nc.tensor.transpose(
                                    ptk[:, i * 128:(i + 1) * 128],
                                    k_nat[scg + i][:, ks], identb)
                            nc.vector.tensor_copy(
                                kT[h][:, scg * 128:(scg + 4) * 128], ptk[:])
                        po = pat.tile([D1, LH], F32, name=f"po{h}", tag="po")
                        for sc in range(NSC):
                            pz = pat.tile([128, LH], F32, name=f"pz{h}_{sc}",
                                          tag="pz", bufs=2)
                            for j in range(2):
                                js = slice(j * 512, (j + 1) * 512)
                                nc.tensor.matmul(
                                    pz[:, js],
                                    kT[h][:, sc * 128:(sc + 1) * 128],
                                    qT[h][:, js], start=True, stop=True)
                            psb = pr.tile([128, LH], BF16,
                                          name=f"psb{h}_{sc}", tag="psb",
                                          bufs=3)
                            nc.scalar.activation(psb[:], pz[:], AF.Exp)
                            for j in range(2):
                                js = slice(j * 512, (j + 1) * 512)
                                nc.tensor.matmul(
                                    po[:, js],
                                    v_sb[sc][:, h * D1:(h + 1) * D1],
                                    psb[:, js], start=(sc == 0),
                                    stop=(sc == NSC - 1))
                        rs = pr.tile([1, LH], F32, name=f"rs{h}", tag="brow",
                                     bufs=4)
                        nc.vector.tensor_copy(rs[:], po[D:D1, :])
                        ri = pr.tile([1, LH], F32, name=f"ri{h}", tag="brow",
                                     bufs=4)
                        nc.vector.reciprocal_approx_fast(ri[:], rs[:])
                        ib = pr.tile([64, LH], F32, name=f"ib{h}", tag="bcol",
                                     bufs=2)
                        nc.gpsimd.partition_broadcast(ib[:], ri[:])
                        nc.vector.tensor_mul(oTp[t][rh:rh + 64, :],
                                             po[0:D, :], ib[:])
                        if debug and h == 0:
                            nc.sync.dma_start(dbg_out["d_kt"][:], kT[0][:])
                            nc.sync.dma_start(dbg_out["d_rs"][:], rs[:])
                            nc.sync.dma_start(dbg_out["d_ri"][:], ri[:])
                            nc.sync.dma_start(dbg_out["d_ib"][:], ib[:])
                        if debug and h == H - 1:
                            nc.sync.dma_start(dbg_out["d_ot"][:],
                                              oTp[0][:].bitcast(F32))

                    for lc in range(NQC):
                        py = scr.tile([128, E], F32, name=f"py{lc}",
                                      tag="scr", bufs=2)
                        for t in range(NT):
                            nc.tensor.matmul(
                                py[:], oTp[t][:, lc * 128:(lc + 1) * 128],
                                wo_t[t][:], start=(t == 0), stop=(t == NT - 1))
                        ysb = pr.tile([128, E], F32, name=f"y{lc}", tag="y",
                                      bufs=2)
                        nc.vector.tensor_add(ysb[:], py[:], bob[:])
                        nc.sync.dma_start(out[lc * 128:(lc + 1) * 128, :],
                                          ysb[:])

        if reps == 1:
            body()
        else:
            ET = mybir.EngineType
            with tc.For_i(0, reps, 1,
                          hint_engines=(ET.PE, ET.Activation, ET.DVE,
                                        ET.Pool, ET.SP)):
                body()
    nc.compile()
    return nc


def _get_nc(reps=1):
    key = f"nc{reps}"
    if key not in _CACHE:
        _CACHE[key] = _build(reps)
    return _CACHE[key]


def _in_maps(query, key, value, Wq, bq, Wk, bk, Wv, bv, Wo, bo):
    maps = []
    for c in range(N_CORES):
        b, half = c // 2, c % 2
        ls = slice(half * LH, (half + 1) * LH)
        maps.append({
            "src_q": np.ascontiguousarray(query[b, ls]),
            "src_k": np.ascontiguousarray(key[b]),
            "src_v": np.ascontiguousarray(value[b]),
            "wq": np.ascontiguousarray(Wq),
            "wk": np.ascontiguousarray(Wk),
            "wv": np.ascontiguousarray(Wv),
            "wo": np.ascontiguousarray(Wo),
            "bq": np.ascontiguousarray(bq).reshape(1, E),
            "bk": np.ascontiguousarray(bk).reshape(1, E),
            "bv": np.ascontiguousarray(bv).reshape(1, E),
            "bo": np.ascontiguousarray(bo).reshape(1, E),
        })
    return maps


def kernel(**inputs):
    inputs = {k: np.asarray(v, dtype=np.float32) for k, v in inputs.items()}
    nc = _get_nc()
    maps = _in_maps(**inputs)
    res = run_bass_kernel_spmd(nc, maps, list(range(N_CORES)))
    out = np.empty((B, S, E), dtype=np.float32)
    for c in range(N_CORES):
        b, half = c // 2, c % 2
        out[b, half * LH:(half + 1) * LH] = res.results[c]["out"]
    _CACHE["last_maps"] = maps
    return out


def _timed_fn(reps):
    """Jitted sharded single-call executable with device-resident buffers."""
    import jax
    from jax.sharding import Mesh, PartitionSpec, NamedSharding
    from jax.experimental.shard_map import shard_map
    from concourse.bass2jax import (_bass_exec_p, partition_id_tensor,
                                    install_neuronx_cc_hook)

    nc = _get_nc(reps)
    install_neuronx_cc_hook()
    in_names, out_names, out_avals = [], [], []
    for alloc in nc.m.functions[0].allocations:
        if not isinstance(alloc, mybir.MemoryLocationSet):
            continue
        name = alloc.memorylocations[0].name
        if alloc.kind == "ExternalInput":
            if name != "partition_id":
                in_names.append(name)
        elif alloc.kind == "ExternalOutput":
            out_names.append(name)
            out_avals.append(jax.core.ShapedArray(
                tuple(alloc.tensor_shape), mybir.dt.np(alloc.dtype)))
    n_params, n_outs = len(in_names), len(out_names)
    all_in = in_names + out_names + ["partition_id"]

    def _body(*args):
        outs = _bass_exec_p.bind(
            *args, partition_id_tensor(),
            out_avals=tuple(out_avals), in_names=tuple(all_in),
            out_names=tuple(out_names), lowering_input_output_aliases=(),
            sim_require_finite=True, sim_require_nnan=True, nc=nc)
        return tuple(outs)

    devices = jax.devices()[:N_CORES]
    mesh = Mesh(np.asarray(devices), ("core",))
    sh = NamedSharding(mesh, PartitionSpec("core"))
    fn = jax.jit(
        shard_map(_body, mesh=mesh,
                  in_specs=(PartitionSpec("core"),) * (n_params + n_outs),
                  out_specs=(PartitionSpec("core"),) * n_outs,
                  check_rep=False),
        keep_unused=True)
    maps = _CACHE["last_maps"]
    darg = [jax.device_put(
                np.concatenate([np.asarray(maps[c][n]) for c in range(N_CORES)],
                               axis=0), sh) for n in in_names]
    darg += [jax.device_put(
                np.zeros((N_CORES * a.shape[0], *a.shape[1:]), a.dtype), sh)
             for a in out_avals]

    def call():
        import jax as _j
        return _j.block_until_ready(fn(*darg))

    return call


def measure_exec_time_ns(reps=128, trials=12):
    """Per-iteration HW time via in-NEFF hardware-loop repetition delta."""
    import time
    call1 = _timed_fn(1)
    callN = _timed_fn(reps)
    call1(); callN()  # warm both executables

    def best(call):
        b = float("inf")
        for _ in range(trials):
            t0 = time.perf_counter()
            call()
            b = min(b, time.perf_counter() - t0)
        return b

    t1, tN = best(call1), best(callN)
    return int((tN - t1) / (reps - 1) * 1e9)


if __name__ == "__main__":
    nc = _get_nc()
    print("built + compiled ok")
